# revision 32
# baseline (speedup 1.0000x reference)
"""Trainium2 Bass kernel for nn_CSGO_model (4-layer transformer + 26 MLP heads).

Sharding: data-parallel over batch (8 batches/core) for the transformer;
tiny bf16 AllGather of x_comb; head-parallel (4 padded head slots/core, 32
slots for 26 heads) for the InvDynamic head MLPs.

Layout: activations are kept feature-major X^T [D, M] on chip (D on
partitions in 128-chunks, M = 8 local batches x 32 timesteps = 256 tokens on
the free dim), so every GEMM is matmul(psum, lhsT=W_chunk, rhs=X_chunk) with
no transposes; V is computed token-major (lhsT = activations) for the AV
matmul. All GEMMs run in bf16 with fp32 PSUM accumulation; the residual
stream, LayerNorm statistics, and softmax stay fp32. LayerNorm column stats
come from all-ones [128,128] matmuls (partition-replicated sums); the LN
affine (g, b) is folded into the following GEMM weights on the host (exact).
Attention computes S^T = K^T.T Q^T over all 256x256 token pairs per head and
adds a rank-5 -800 block mask inside the PSUM accumulation so exp() zeroes
cross-batch pairs (8x FLOP waste, but T=32 is tiny and it keeps every matmul
dense).
"""
import sys
import os
import types

sys.path.insert(0, '/opt/trn_rl_repo')

# bass_utils imports antenv.axon_hooks when BASS_TRACE is set; that module
# does not exist in this image, so install a no-op shim defensively.
if 'antenv.axon_hooks' not in sys.modules:
    try:
        from antenv import axon_hooks  # noqa: F401
    except ImportError:
        _hookmod = types.ModuleType('antenv.axon_hooks')
        _hookmod.set_axon_ntff_profile_hook = lambda h: None
        _hookmod.get_axon_ntff_profile_hook = lambda: None
        sys.modules['antenv.axon_hooks'] = _hookmod

import numpy as np
import ml_dtypes

BF16 = ml_dtypes.bfloat16

# Model dims
D = 1024
NHEADS = 16
HD = 64
INNER = NHEADS * HD
FF = 2048
L = 4
NOUT = 26
IDH = 512
B = 64
T = 32

N_CORES = 8
B_LOC = B // N_CORES          # 8 batches per core
M = B_LOC * T                 # 256 tokens per core
DCH = D // 128                # 8 feature chunks
FCH = FF // 128               # 16
H_SLOTS = 4                   # padded head slots per core (8*4=32 >= 26)

_CACHE = {}


# ---------------------------------------------------------------- device code

def _build_nc():
    import concourse.tile as tile
    from concourse import mybir, bacc

    f32 = mybir.dt.float32
    f32r = mybir.dt.float32r
    bf16 = mybir.dt.bfloat16
    Alu = mybir.AluOpType
    Act = mybir.ActivationFunctionType

    nc = bacc.Bacc("TRN2", target_bir_lowering=False, debug=False,
                   num_devices=N_CORES)

    # ------------- DRAM tensors (per-core inputs, host-prepared layouts)
    x_d = nc.dram_tensor("x", [DCH, 128, M], f32, kind="ExternalInput")
    wq_d = nc.dram_tensor("wq", [L, DCH, 128, 3 * INNER], bf16,
                          kind="ExternalInput")
    wo_d = nc.dram_tensor("wo", [L, DCH, 128, D], bf16, kind="ExternalInput")
    wf1_d = nc.dram_tensor("wf1", [L, DCH, 128, FF], bf16,
                           kind="ExternalInput")
    wf2_d = nc.dram_tensor("wf2", [L, FCH, 128, D], bf16,
                           kind="ExternalInput")
    biasp_d = nc.dram_tensor("biasp", [128, L, 32], f32, kind="ExternalInput")
    cpack_d = nc.dram_tensor("cpack", [128, 1024], bf16, kind="ExternalInput")
    qkbp_d = nc.dram_tensor("qkbp", [128, L, 2 * DCH], f32,
                            kind="ExternalInput")
    cpack32_d = nc.dram_tensor("cpack32", [128, 128], f32,
                               kind="ExternalInput")
    hw1_d = nc.dram_tensor("hw1", [H_SLOTS, 2 * DCH, 128, IDH], bf16,
                           kind="ExternalInput")
    hw2_d = nc.dram_tensor("hw2", [H_SLOTS, 4, 128, IDH], bf16,
                           kind="ExternalInput")
    hw3p_d = nc.dram_tensor("hw3p", [128, H_SLOTS * 4], bf16,
                            kind="ExternalInput")
    hbrow_d = nc.dram_tensor("hbrow", [1, H_SLOTS, 2 * IDH], bf16,
                             kind="ExternalInput")
    hbp_d = nc.dram_tensor("hbp", [128, H_SLOTS, 1], f32,
                           kind="ExternalInput")

    out_d = nc.dram_tensor("out_h", [B, H_SLOTS], f32, kind="ExternalOutput")
    xcg_d = nc.dram_tensor("xcg", [N_CORES * 128, 128], bf16,
                           kind="ExternalOutput")

    with tile.TileContext(nc) as tc:
        from contextlib import ExitStack
        with ExitStack() as ctx:
            const = ctx.enter_context(tc.tile_pool(name="const", bufs=1))
            ps_a = ctx.enter_context(
                tc.tile_pool(name="ps_a", bufs=4, space="PSUM"))
            ps_op = ctx.enter_context(
                tc.tile_pool(name="ps_op", bufs=4, space="PSUM"))
            dram = ctx.enter_context(
                tc.tile_pool(name="dram", bufs=1, space="DRAM"))
            tfs = ctx.enter_context(ExitStack())
            hres = tfs.enter_context(tc.tile_pool(name="hres", bufs=1))
            sq = tfs.enter_context(tc.tile_pool(name="sq", bufs=4))
            stats = tfs.enter_context(tc.tile_pool(name="stats", bufs=6))
            actb = tfs.enter_context(tc.tile_pool(name="actb", bufs=2))
            qkp = tfs.enter_context(tc.tile_pool(name="qkp", bufs=1))
            vtokp = tfs.enter_context(tc.tile_pool(name="vtokp", bufs=1))
            attp = tfs.enter_context(tc.tile_pool(name="attp", bufs=6))
            obufp = tfs.enter_context(tc.tile_pool(name="obufp", bufs=1))
            g1p = tfs.enter_context(tc.tile_pool(name="g1p", bufs=1))

            # constants
            cpack = const.tile([128, 1024], bf16, tag="cpack")
            nc.sync.dma_start(cpack[:], cpack_d[:])
            ident = cpack[:, 0:128]        # identity (bf16)
            jones = cpack[:, 128:256]      # all-ones (bf16)
            mrow = cpack[0:5, 384:512]     # [5,128] mask lhsT
            mcols = [cpack[0:5, 512:768], cpack[0:5, 768:1024]]  # [5,256] x2
            cpack32 = const.tile([128, 128], f32, tag="cpack32")
            nc.sync.dma_start(cpack32[:], cpack32_d[:])
            jones32 = cpack32[:, 0:128]    # all-ones (f32)
            qkbp = const.tile([128, L, 2 * DCH], f32, tag="qkbp")
            nc.sync.dma_start(qkbp[:], qkbp_d[:])
            biasp = const.tile([128, L, 32], f32, tag="biasp")
            nc.sync.dma_start(biasp[:], biasp_d[:])
            # LN epsilons as [128,1] const APs (float act-bias needs an AP)
            eps0 = const.tile([128, 1], f32, tag="eps0")
            nc.vector.memset(eps0[:], 1e-6)
            eps1 = const.tile([128, 1], f32, tag="eps1")
            nc.vector.memset(eps1[:], 1e-5)

            # residual, feature-major [128p, chunk, token], fp32
            h = hres.tile([128, DCH, M], f32, tag="h")
            nc.sync.dma_start(h[:], x_d[:].rearrange("c p m -> p c m"))

            def layer_norm(l, site):
                """Returns xln_bf [128, DCH, M] bf16.

                site 0: collapsed double-LN (no-affine eps=1e-6 then affine
                eps=1e-5)  -> rsqrt(v*(1+1e-5) + 1.00001e-6), then *g+b.
                site 1: plain affine LN eps=1e-5 -> rsqrt(v + 1e-5).
                """
                # column sums (fp32 matmul, no cast needed) and
                # sum-of-squares (bf16 via ACT Square) via all-ones matmuls
                ps_s = ps_a.tile([128, M], f32, tag="a")
                ps_q = ps_a.tile([128, M], f32, tag="a")
                for c in range(DCH):
                    nc.tensor.matmul(ps_s[:], jones32, h[:, c, :],
                                     start=(c == 0), stop=(c == DCH - 1))
                for c in range(DCH):
                    hsq = sq.tile([128, M], bf16, tag="sq")
                    nc.scalar.activation(hsq[:], h[:, c, :], Act.Square)
                    nc.tensor.matmul(ps_q[:], jones, hsq[:],
                                     start=(c == 0), stop=(c == DCH - 1))
                mu = stats.tile([128, M], f32, tag="st")
                nc.vector.tensor_scalar_mul(mu[:], ps_s[:], 1.0 / D)
                mu2 = stats.tile([128, M], f32, tag="st")
                nc.vector.tensor_mul(mu2[:], mu[:], mu[:])
                v = stats.tile([128, M], f32, tag="st")
                # v = ps_q/D - mu^2  (one fused op)
                nc.vector.scalar_tensor_tensor(v[:], ps_q[:], 1.0 / D, mu2[:],
                                               Alu.mult, Alu.subtract)
                # site 0 collapsed double-LN: rsqrt(v*(1+1e-5) + 1.00001e-6)
                #   = rsqrt(v + 1e-6) / sqrt(1+1e-5); the 1/sqrt(1+1e-5) is
                #   folded into the host-side gain g.
                alpha = stats.tile([128, M], f32, tag="st")
                sd = stats.tile([128, M], f32, tag="st")
                nc.scalar.activation(sd[:], v[:], Act.Sqrt,
                                     bias=(eps0 if site == 0 else eps1)[:])
                nc.vector.reciprocal_approx_fast(alpha[:], sd[:])
                xln = actb.tile([128, DCH, M], bf16, tag="xln")
                for cs in ((0, 1), (1, 3), (3, 5), (5, 7), (7, 8)):
                    c0, c1 = cs
                    w = c1 - c0
                    tt_full = sq.tile([128, 2, M], f32, tag="sqf")
                    t = tt_full[:, :w, :]
                    mu_b = mu[:].unsqueeze(1).to_broadcast([128, w, M])
                    al_b = alpha[:].unsqueeze(1).to_broadcast([128, w, M])
                    nc.vector.tensor_sub(t[:], h[:, c0:c1, :], mu_b)
                    nc.vector.tensor_mul(xln[:, c0:c1, :], t[:], al_b)
                return xln

            wq_pool = tfs.enter_context(tc.tile_pool(name="wq", bufs=1))
            wo_pool = tfs.enter_context(tc.tile_pool(name="wo", bufs=1))
            wf1_pool = tfs.enter_context(tc.tile_pool(name="wf1", bufs=1))
            wf2_pool = tfs.enter_context(tc.tile_pool(name="wf2", bufs=1))

            for l in range(L):
                # ---- attn pre-LN (collapsed double LN)
                xln = layer_norm(l, 0)

                wq = wq_pool.tile([128, DCH, 3 * INNER], bf16, tag="wq")
                for c in range(DCH):
                    nc.sync.dma_start(wq[:, c, :], wq_d[l, c])

                # ---- Q,K feature-major GEMM: out [2*INNER, M]
                qk = qkp.tile([128, 2 * DCH, M], bf16, tag="qk")
                for np_ in range(DCH):
                    ps = ps_a.tile([128, 2, M], f32, tag="a")
                    for i in range(2):
                        n = 2 * np_ + i
                        for c in range(DCH):
                            nc.tensor.matmul(ps[:, i, :],
                                             wq[:, c, n * 128:(n + 1) * 128],
                                             xln[:, c, :],
                                             start=(c == 0),
                                             stop=(c == DCH - 1))
                    for i in range(2):
                        n = 2 * np_ + i
                        nc.scalar.activation(qk[:, n, :], ps[:, i, :],
                                             Act.Identity,
                                             bias=qkbp[:, l, n:n + 1])

                # ---- V token-major GEMM: out [M, INNER]
                vtok = vtokp.tile([128, 2, INNER], bf16, tag="vtok")
                for mc in range(2):
                    for ns in range(2):
                        psv = ps_a.tile([128, 512], f32, tag="a")
                        for c in range(DCH):
                            nc.tensor.matmul(
                                psv[:],
                                xln[:, c, mc * 128:(mc + 1) * 128],
                                wq[:, c, 2 * INNER + ns * 512:
                                   2 * INNER + (ns + 1) * 512],
                                start=(c == 0), stop=(c == DCH - 1))
                        nc.scalar.activation(
                            vtok[:, mc, ns * 512:(ns + 1) * 512], psv[:],
                            Act.Copy)

                # ---- attention, head-pair by head-pair, with outproj
                # pass A (obuf chunks 0..3) interleaved at pairs 4..7 to keep
                # the PE dense and warm. Each PSUM accumulation group starts
                # AND stops within its pass, so groups stay sequential within
                # every bank (interleaved groups in one bank corrupt).
                wo = wo_pool.tile([128, DCH, D], bf16, tag="wo")
                nc.sync.dma_start(wo[:], wo_d[l].rearrange("c p n -> p c n"))
                op_ps = [ps_op.tile([128, 2, M], f32, tag="op",
                                    name=f"op_{l}_{j}") for j in range(4)]

                def outproj_pass(np_, c0, c1, with_bias):
                    for i in range(2):
                        n = 2 * np_ + i
                        for c in range(c0, c1):
                            nc.tensor.matmul(
                                op_ps[np_][:, i, :],
                                wo[:, c, n * 128:(n + 1) * 128],
                                obuf[:, c, :],
                                start=(c == c0), stop=(c == c1 - 1))
                    for i in range(2):
                        n = 2 * np_ + i
                        if with_bias:
                            nc.vector.scalar_tensor_tensor(
                                h[:, n, :], op_ps[np_][:, i, :],
                                biasp[:, l, n:n + 1], h[:, n, :],
                                Alu.add, Alu.add)
                        else:
                            nc.vector.tensor_add(h[:, n, :], h[:, n, :],
                                                 op_ps[np_][:, i, :])

                obuf = obufp.tile([128, DCH, M], bf16, tag="obuf")
                for hc in range(DCH):
                    # head pair (2*hc, 2*hc+1): even at partitions 0:64,
                    # odd at 64:128 -> S matmuls interleave across row
                    # groups so the PE array runs them concurrently.
                    es = []
                    pss = []
                    for j in range(2):
                        e_j = attp.tile([128, 2, M], bf16, tag="e",
                                        name=f"e_{l}_{hc}_{j}")
                        ps_j = ps_a.tile([128, 2, M], f32, tag="a",
                                         name=f"pss_{l}_{hc}_{j}")
                        es.append(e_j)
                        pss.append(ps_j)
                    for i in range(2):
                        for j in range(2):
                            hp = j * 64
                            nc.tensor.matmul(
                                pss[j][:, i, :],
                                qk[hp:hp + 64, DCH + hc,
                                   i * 128:(i + 1) * 128],
                                qk[hp:hp + 64, hc, :],
                                start=True, stop=False)
                        for j in range(2):
                            nc.tensor.matmul(pss[j][:, i, :], mrow, mcols[i],
                                             start=False, stop=True)
                    for j in range(2):
                        nc.scalar.activation(es[j][:], pss[j][:], Act.Exp,
                                             scale=0.125)
                    for j in range(2):
                        hh = 2 * hc + j
                        hp = j * 64
                        e = es[j]
                        ps_dn = ps_a.tile([128, M], f32, tag="a")
                        for i in range(2):
                            nc.tensor.matmul(ps_dn[:], jones, e[:, i, :],
                                             start=(i == 0), stop=(i == 1))
                        rd = stats.tile([128, M], f32, tag="st")
                        nc.vector.reciprocal_approx_fast(rd[0:64, :],
                                                         ps_dn[0:64, :])
                        ps_o = ps_a.tile([128, M], f32, tag="a")
                        for i in range(2):
                            nc.tensor.matmul(
                                ps_o[hp:hp + 64, :],
                                vtok[:, i, hh * 64:(hh + 1) * 64],
                                e[:, i, :],
                                start=(i == 0), stop=(i == 1),
                                tile_position=(0, hp))
                        nc.vector.tensor_tensor(
                            obuf[hp:hp + 64, hc, :], ps_o[hp:hp + 64, :],
                            rd[0:64, :], Alu.mult)
                # ---- output projection + residual + out_b
                wo = wo_pool.tile([128, DCH, D], bf16, tag="wo")
                nc.sync.dma_start(wo[:], wo_d[l].rearrange("c p n -> p c n"))
                for n in range(DCH):
                    ps = ps_a.tile([128, M], f32, tag="a")
                    for c in range(DCH):
                        nc.tensor.matmul(ps[:],
                                         wo[:, c, n * 128:(n + 1) * 128],
                                         obuf[:, c, :],
                                         start=(c == 0), stop=(c == DCH - 1))
                    nc.vector.scalar_tensor_tensor(
                        h[:, n, :], ps[:], biasp[:, l, n:n + 1], h[:, n, :],
                        Alu.add, Alu.add)

                # ---- ff pre-LN
                xln2 = layer_norm(l, 1)

                # ---- ff1 + gelu(x + b1)
                wf1 = wf1_pool.tile([128, DCH, FF], bf16, tag="wf1")
                nc.sync.dma_start(wf1[:], wf1_d[l].rearrange("c p n -> p c n"))
                g1 = g1p.tile([128, FCH, M], bf16, tag="g1")
                for n in range(FCH):
                    ps = ps_a.tile([128, M], f32, tag="a")
                    for c in range(DCH):
                        nc.tensor.matmul(ps[:],
                                         wf1[:, c, n * 128:(n + 1) * 128],
                                         xln2[:, c, :],
                                         start=(c == 0), stop=(c == DCH - 1))
                    nc.scalar.activation(g1[:, n, :], ps[:], Act.Gelu,
                                         bias=biasp[:, l, 8 + n:9 + n])

                # ---- ff2 + residual + b2
                wf2 = wf2_pool.tile([128, FCH, D], bf16, tag="wf2")
                nc.sync.dma_start(wf2[:], wf2_d[l].rearrange("c p n -> p c n"))
                for n in range(DCH):
                    ps = ps_a.tile([128, M], f32, tag="a")
                    for c in range(FCH):
                        nc.tensor.matmul(ps[:],
                                         wf2[:, c, n * 128:(n + 1) * 128],
                                         g1[:, c, :],
                                         start=(c == 0), stop=(c == FCH - 1))
                    nc.vector.scalar_tensor_tensor(
                        h[:, n, :], ps[:], biasp[:, l, 24 + n:25 + n],
                        h[:, n, :], Alu.add, Alu.add)

            # ---------------- AllGather x_comb ----------------
            # local contribution: columns m = b*32 + t for t in {0,1}
            x16 = const.tile([128, DCH, 2, B_LOC], bf16, tag="x16")
            src = h[:].rearrange("p c (b tt) -> p c tt b", b=B_LOC)[:, :, 0:2, :]
            nc.vector.tensor_copy(x16[:], src)
            tfs.close()   # free transformer pools for the heads stage
            cc_in = dram.tile([128, 128], bf16)
            nc.sync.dma_start(cc_in[:], x16[:].rearrange("p c tt b -> p (c tt b)"))
            cc_out = dram.tile([N_CORES * 128, 128], bf16)
            nc.gpsimd.collective_compute(
                "AllGather", Alu.bypass,
                replica_groups=[list(range(N_CORES))],
                ins=[cc_in[:].opt()], outs=[cc_out[:].opt()])
            nc.sync.dma_start(xcg_d[:], cc_out[:])

            gsb = const.tile([128, N_CORES, 128], bf16, tag="gsb")
            nc.sync.dma_start(
                gsb[:], cc_out[:].rearrange("(j p) f -> p j f", p=128))
            # PE warm-up after the long AllGather idle gap: a few throwaway
            # matmuls on the freshly-reloaded buffer re-arm the HAM clock
            # before the timing-critical head GEMMs.
            ps_w = ps_a.tile([128, 2, M], f32, tag="a", name="warmup_ps")
            for wi in range(16):
                nc.tensor.matmul(ps_w[:, wi % 2, :],
                                 gsb[:, wi % N_CORES, :],
                                 gsb[:].rearrange("p j f -> p (j f)")[:, 0:M],
                                 start=(wi < 2), stop=(wi >= 14))

            # build lhsT x_comb^T [2D, B] as bf16 [128, 16, 64]
            # gsb free layout per core j: (c, tt, b); feature chunk kc of
            # x_comb^T = tt*DCH + c
            xcombT = const.tile([128, 2 * DCH, B], bf16, tag="xcombT")
            for kc in range(2 * DCH):
                tt, c = kc // DCH, kc % DCH
                nc.vector.tensor_copy(
                    xcombT[:, kc, :],
                    gsb[:, :, c * 16 + tt * 8: c * 16 + tt * 8 + 8])

            # ---------------- 26 (padded 32) MLP heads ----------------
            hbb = const.tile([64, H_SLOTS, 2 * IDH], bf16, tag="hbb")
            import concourse.bass as bass_mod
            hb_bcast = bass_mod.AP(
                tensor=hbrow_d[:].tensor, offset=hbrow_d[:].offset,
                ap=[[0, 64]] + hbrow_d[:].ap[1:])
            nc.sync.dma_start(hbb[:], hb_bcast)
            hbp = const.tile([128, H_SLOTS, 1], f32, tag="hbp")
            nc.sync.dma_start(hbp[:], hbp_d[:])
            hw3 = const.tile([128, H_SLOTS * 4], bf16, tag="hw3")
            nc.sync.dma_start(hw3[:], hw3p_d[:])
            outacc = const.tile([64, H_SLOTS], f32, tag="outacc")

            w1h_pool = ctx.enter_context(tc.tile_pool(name="w1h", bufs=2))
            w2h_pool = ctx.enter_context(tc.tile_pool(name="w2h", bufs=2))
            hact = ctx.enter_context(tc.tile_pool(name="hact", bufs=2))

            # prefetch head weights (independent of the AllGather)
            w1h_tiles, w2h_tiles = [], []
            for n in range(H_SLOTS):
                w1h = w1h_pool.tile([128, 2 * DCH, IDH], bf16, tag="w1h")
                nc.sync.dma_start(w1h[:],
                                  hw1_d[n].rearrange("c p n2 -> p c n2"))
                w2h = w2h_pool.tile([128, 4, IDH], bf16, tag="w2h")
                nc.sync.dma_start(w2h[:],
                                  hw2_d[n].rearrange("c p n2 -> p c n2"))
                w1h_tiles.append(w1h)
                w2h_tiles.append(w2h)

            for n in range(H_SLOTS):
                w1h = w1h_tiles[n]
                w2h = w2h_tiles[n]
                ps1 = ps_a.tile([64, IDH], f32, tag="a")
                for kc in range(2 * DCH):
                    nc.tensor.matmul(ps1[:], xcombT[:, kc, :], w1h[:, kc, :],
                                     start=(kc == 0), stop=(kc == 2 * DCH - 1))
                t1 = hact.tile([64, IDH], f32, tag="ht")
                nc.vector.tensor_tensor(t1[:], ps1[:], hbb[:, n, 0:IDH],
                                        Alu.add)
                h1 = hact.tile([64, IDH], bf16, tag="hb")
                nc.vector.tensor_relu(h1[:], t1[:])
                # transpose h1 -> [IDH, 64]
                h1t = hact.tile([128, 4, 64], bf16, tag="h1t")
                for j in range(4):
                    pst = ps_a.tile([128, 64], bf16, tag="a")
                    nc.tensor.transpose(pst[:], h1[:, j * 128:(j + 1) * 128],
                                        ident[0:64, 0:64])
                    nc.vector.tensor_copy(h1t[:, j, :], pst[:])
                ps2 = ps_a.tile([64, IDH], f32, tag="a")
                for kc in range(4):
                    nc.tensor.matmul(ps2[:], h1t[:, kc, :], w2h[:, kc, :],
                                     start=(kc == 0), stop=(kc == 3))
                t2 = hact.tile([64, IDH], f32, tag="ht")
                nc.vector.tensor_tensor(t2[:], ps2[:], hbb[:, n, IDH:2 * IDH],
                                        Alu.add)
                h2 = hact.tile([64, IDH], bf16, tag="hb")
                nc.vector.tensor_relu(h2[:], t2[:])
                h2t = hact.tile([128, 4, 64], bf16, tag="h2t")
                for j in range(4):
                    pst = ps_a.tile([128, 64], bf16, tag="a")
                    nc.tensor.transpose(pst[:], h2[:, j * 128:(j + 1) * 128],
                                        ident[0:64, 0:64])
                    nc.vector.tensor_copy(h2t[:, j, :], pst[:])
                ps3 = ps_a.tile([64, 1], f32, tag="a")
                for kc in range(4):
                    nc.tensor.matmul(ps3[:], h2t[:, kc, :],
                                     hw3[:, n * 4 + kc:n * 4 + kc + 1],
                                     start=(kc == 0), stop=(kc == 3))
                nc.vector.tensor_scalar_add(outacc[:, n:n + 1], ps3[:],
                                            hbp[0:64, n, :])

            nc.sync.dma_start(out_d[:], outacc[:])

    nc.finalize()
    return nc


# ---------------------------------------------------------------- host side

def _head_map():
    """global head g -> (core, slot); core = g % 8, slot = g // 8."""
    m = []
    for n in range(H_SLOTS):
        for c in range(N_CORES):
            m.append(n * N_CORES + c)  # slot-major global index
    return m


def _prep_in_maps(inputs):
    x = np.asarray(inputs['x'], np.float32)
    qkv_w = np.asarray(inputs['qkv_w'], np.float32)
    out_w = np.asarray(inputs['out_w'], np.float32)
    out_b = np.asarray(inputs['out_b'], np.float32)
    attn_ln_g = np.asarray(inputs['attn_ln_g'], np.float32)
    attn_ln_b = np.asarray(inputs['attn_ln_b'], np.float32)
    ff_ln_g = np.asarray(inputs['ff_ln_g'], np.float32)
    ff_ln_b = np.asarray(inputs['ff_ln_b'], np.float32)
    ff_w1 = np.asarray(inputs['ff_w1'], np.float32)
    ff_b1 = np.asarray(inputs['ff_b1'], np.float32)
    ff_w2 = np.asarray(inputs['ff_w2'], np.float32)
    ff_b2 = np.asarray(inputs['ff_b2'], np.float32)
    head_w1 = np.asarray(inputs['head_w1'], np.float32)
    head_b1 = np.asarray(inputs['head_b1'], np.float32)
    head_w2 = np.asarray(inputs['head_w2'], np.float32)
    head_b2 = np.asarray(inputs['head_b2'], np.float32)
    head_w3 = np.asarray(inputs['head_w3'], np.float32)
    head_b3 = np.asarray(inputs['head_b3'], np.float32)

    # Fold the LN affine transform into the following GEMM weights (exact):
    #   xln = (h-mu)*alpha_hat;  y = (xln*g + b) @ W = xln @ (diag(g) W) + b@W
    # The collapsed double-LN 1/sqrt(1+1e-5) factor is folded into g too.
    ag_eff = attn_ln_g * np.float32((1.0 + 1e-5) ** -0.5)   # [L, D]
    qkvb = np.einsum('ld,ldn->ln', attn_ln_b, qkv_w)        # [L, 3*INNER]
    ff_b1 = ff_b1 + np.einsum('ld,ldn->ln', ff_ln_b, ff_w1)
    qkv_w = qkv_w * ag_eff[:, :, None]
    ff_w1 = ff_w1 * ff_ln_g[:, :, None]
    # V's LN-bias contribution passes through softmax unchanged (weights sum
    # to 1), so it folds into the output-projection bias exactly.
    vbias = qkvb[:, 2 * INNER:]                              # [L, INNER]
    out_b = out_b + np.einsum('lk,lkd->ld', vbias, out_w)

    # shared (replicated) tensors
    wq = np.ascontiguousarray(
        qkv_w.reshape(L, DCH, 128, 3 * INNER)).astype(BF16)
    wo = np.ascontiguousarray(out_w.reshape(L, DCH, 128, D)).astype(BF16)
    wf1 = np.ascontiguousarray(ff_w1.reshape(L, DCH, 128, FF)).astype(BF16)
    wf2 = np.ascontiguousarray(ff_w2.reshape(L, FCH, 128, D)).astype(BF16)

    biasp = np.zeros((128, L, 32), np.float32)
    biasp[:, :, 0:8] = out_b.reshape(L, 8, 128).transpose(2, 0, 1)
    biasp[:, :, 8:24] = ff_b1.reshape(L, 16, 128).transpose(2, 0, 1)
    biasp[:, :, 24:32] = ff_b2.reshape(L, 8, 128).transpose(2, 0, 1)

    qkbp = np.ascontiguousarray(
        qkvb[:, :2 * INNER].reshape(L, 2 * DCH, 128).transpose(2, 0, 1))

    cpack = np.zeros((128, 1024), np.float32)
    cpack[:, 0:128] = np.eye(128, dtype=np.float32)
    cpack[:, 128:384] = 1.0
    # rank-5 additive attention mask: M_i = -800*J + 800*sum_bl u_bl (x) v_{4i+bl}
    # (applied inside the S-matmul PSUM accumulation; exp(0.125*-800) == 0)
    cpack[0, 384:512] = 1.0                       # ones row of lhsT
    for bl in range(4):
        cpack[1 + bl, 384 + 32 * bl:384 + 32 * (bl + 1)] = 1.0   # u_bl
    for i in range(2):
        base = 512 + 256 * i
        cpack[0, base:base + 256] = -800.0
        for bl in range(4):
            bk = 4 * i + bl
            cpack[1 + bl, base + 32 * bk:base + 32 * (bk + 1)] = 800.0
    cpack = cpack.astype(BF16)
    cpack32 = np.ones((128, 128), np.float32)

    in_maps = []
    for c in range(N_CORES):
        xs = x[c * B_LOC:(c + 1) * B_LOC].reshape(M, D)  # [256, 1024]
        x_fm = np.ascontiguousarray(xs.T.reshape(DCH, 128, M))

        hw1 = np.zeros((H_SLOTS, 2 * DCH, 128, IDH), np.float32)
        hw2 = np.zeros((H_SLOTS, 4, 128, IDH), np.float32)
        hw3p = np.zeros((128, H_SLOTS * 4), np.float32)
        hbrow = np.zeros((1, H_SLOTS, 2 * IDH), np.float32)
        hbp = np.zeros((128, H_SLOTS, 1), np.float32)
        for n in range(H_SLOTS):
            g = n * N_CORES + c
            if g >= NOUT:
                continue
            hw1[n] = head_w1[g].reshape(2 * DCH, 128, IDH)
            hw2[n] = head_w2[g].reshape(4, 128, IDH)
            hw3p[:, n * 4:(n + 1) * 4] = head_w3[g].reshape(4, 128).T
            hbrow[0, n, 0:IDH] = head_b1[g]
            hbrow[0, n, IDH:2 * IDH] = head_b2[g]
            hbp[:, n, 0] = head_b3[g, 0]
        in_maps.append({
            'x': x_fm,
            'wq': wq, 'wo': wo, 'wf1': wf1, 'wf2': wf2,
            'biasp': biasp, 'qkbp': qkbp,
            'cpack': cpack, 'cpack32': cpack32,
            'hw1': hw1.astype(BF16), 'hw2': hw2.astype(BF16),
            'hw3p': hw3p.astype(BF16), 'hbrow': hbrow.astype(BF16),
            'hbp': hbp,
        })
    return in_maps


def _get_nc():
    if 'nc' not in _CACHE:
        _CACHE['nc'] = _build_nc()
    return _CACHE['nc']


def kernel(**inputs):
    from concourse.bass_utils import run_bass_kernel_spmd
    nc = _get_nc()
    in_maps = _prep_in_maps(inputs)
    res = run_bass_kernel_spmd(nc, in_maps, core_ids=list(range(N_CORES)))
    out = np.zeros((B, NOUT, 1), np.float32)
    for c in range(N_CORES):
        oh = res.results[c]['out_h']       # [64, H_SLOTS]
        for n in range(H_SLOTS):
            g = n * N_CORES + c
            if g < NOUT:
                out[:, g, 0] = oh[:, n]
    return out


# revision 33
# speedup vs baseline: 1.4725x; 1.4725x over previous
"""Trainium2 Bass kernel for nn_CSGO_model (4-layer transformer + 26 MLP heads).

Sharding: data-parallel over batch (8 batches/core) for the transformer;
tiny bf16 AllGather of x_comb; head-parallel (4 padded head slots/core, 32
slots for 26 heads) for the InvDynamic head MLPs.

Layout: activations are kept feature-major X^T [D, M] on chip (D on
partitions in 128-chunks, M = 8 local batches x 32 timesteps = 256 tokens on
the free dim), so every GEMM is matmul(psum, lhsT=W_chunk, rhs=X_chunk) with
no transposes; V is computed token-major (lhsT = activations) for the AV
matmul. All GEMMs run in bf16 with fp32 PSUM accumulation; the residual
stream, LayerNorm statistics, and softmax stay fp32. LayerNorm column stats
come from all-ones [128,128] matmuls (partition-replicated sums); the LN
affine (g, b) is folded into the following GEMM weights on the host (exact).
Attention computes S^T = K^T.T Q^T over all 256x256 token pairs per head and
adds a rank-5 -800 block mask inside the PSUM accumulation so exp() zeroes
cross-batch pairs (8x FLOP waste, but T=32 is tiny and it keeps every matmul
dense).
"""
import sys
import os
import types

sys.path.insert(0, '/opt/trn_rl_repo')

# bass_utils imports antenv.axon_hooks when BASS_TRACE is set; that module
# does not exist in this image, so install a no-op shim defensively.
if 'antenv.axon_hooks' not in sys.modules:
    try:
        from antenv import axon_hooks  # noqa: F401
    except ImportError:
        _hookmod = types.ModuleType('antenv.axon_hooks')
        _hookmod.set_axon_ntff_profile_hook = lambda h: None
        _hookmod.get_axon_ntff_profile_hook = lambda: None
        sys.modules['antenv.axon_hooks'] = _hookmod

import numpy as np
import ml_dtypes

BF16 = ml_dtypes.bfloat16

# Model dims
D = 1024
NHEADS = 16
HD = 64
INNER = NHEADS * HD
FF = 2048
L = 4
NOUT = 26
IDH = 512
B = 64
T = 32

N_CORES = 8
B_LOC = B // N_CORES          # 8 batches per core
M = B_LOC * T                 # 256 tokens per core
DCH = D // 128                # 8 feature chunks
FCH = FF // 128               # 16
H_SLOTS = 4                   # padded head slots per core (8*4=32 >= 26)

_CACHE = {}


# ---------------------------------------------------------------- device code

def _build_nc():
    import concourse.tile as tile
    from concourse import mybir, bacc

    f32 = mybir.dt.float32
    f32r = mybir.dt.float32r
    bf16 = mybir.dt.bfloat16
    Alu = mybir.AluOpType
    Act = mybir.ActivationFunctionType

    nc = bacc.Bacc("TRN2", target_bir_lowering=False, debug=False,
                   num_devices=N_CORES)

    # ------------- DRAM tensors (per-core inputs, host-prepared layouts)
    x_d = nc.dram_tensor("x", [DCH, 128, M], f32, kind="ExternalInput")
    wq_d = nc.dram_tensor("wq", [L, DCH, 128, 3 * INNER], bf16,
                          kind="ExternalInput")
    wo_d = nc.dram_tensor("wo", [L, DCH, 128, D], bf16, kind="ExternalInput")
    wf1_d = nc.dram_tensor("wf1", [L, DCH, 128, FF], bf16,
                           kind="ExternalInput")
    wf2_d = nc.dram_tensor("wf2", [L, FCH, 128, D], bf16,
                           kind="ExternalInput")
    biasp_d = nc.dram_tensor("biasp", [128, L, 32], f32, kind="ExternalInput")
    cpack_d = nc.dram_tensor("cpack", [128, 1024], bf16, kind="ExternalInput")
    qkbp_d = nc.dram_tensor("qkbp", [128, L, 2 * DCH], f32,
                            kind="ExternalInput")
    cpack32_d = nc.dram_tensor("cpack32", [128, 128], f32,
                               kind="ExternalInput")
    hw1_d = nc.dram_tensor("hw1", [H_SLOTS, 2 * DCH, 128, IDH], bf16,
                           kind="ExternalInput")
    hw2_d = nc.dram_tensor("hw2", [H_SLOTS, 4, 128, IDH], bf16,
                           kind="ExternalInput")
    hw3p_d = nc.dram_tensor("hw3p", [128, H_SLOTS * 4], bf16,
                            kind="ExternalInput")
    hbrow_d = nc.dram_tensor("hbrow", [1, H_SLOTS, 2 * IDH], bf16,
                             kind="ExternalInput")
    hbp_d = nc.dram_tensor("hbp", [128, H_SLOTS, 1], f32,
                           kind="ExternalInput")

    out_d = nc.dram_tensor("out_h", [B, H_SLOTS], f32, kind="ExternalOutput")
    xcg_d = nc.dram_tensor("xcg", [N_CORES * 128, 128], bf16,
                           kind="ExternalOutput")

    with tile.TileContext(nc) as tc:
        from contextlib import ExitStack
        with ExitStack() as ctx:
            const = ctx.enter_context(tc.tile_pool(name="const", bufs=1))
            ps_a = ctx.enter_context(
                tc.tile_pool(name="ps_a", bufs=7, space="PSUM"))
            ps_v = ctx.enter_context(
                tc.tile_pool(name="ps_v", bufs=1, space="PSUM"))
            dram = ctx.enter_context(
                tc.tile_pool(name="dram", bufs=1, space="DRAM"))
            tfs = ctx.enter_context(ExitStack())
            hres = tfs.enter_context(tc.tile_pool(name="hres", bufs=1))
            sq = tfs.enter_context(tc.tile_pool(name="sq", bufs=4))
            stats = tfs.enter_context(tc.tile_pool(name="stats", bufs=6))
            actb = tfs.enter_context(tc.tile_pool(name="actb", bufs=2))
            qkp = tfs.enter_context(tc.tile_pool(name="qkp", bufs=1))
            vtokp = tfs.enter_context(tc.tile_pool(name="vtokp", bufs=1))
            attp = tfs.enter_context(tc.tile_pool(name="attp", bufs=6))
            obufp = tfs.enter_context(tc.tile_pool(name="obufp", bufs=1))
            g1p = tfs.enter_context(tc.tile_pool(name="g1p", bufs=1))

            # constants
            cpack = const.tile([128, 1024], bf16, tag="cpack")
            nc.sync.dma_start(cpack[:], cpack_d[:])
            ident = cpack[:, 0:128]        # identity (bf16)
            jones = cpack[:, 128:256]      # all-ones (bf16)
            mrow = cpack[0:5, 384:512]     # [5,128] mask lhsT
            mcols = [cpack[0:5, 512:768], cpack[0:5, 768:1024]]  # [5,256] x2
            cpack32 = const.tile([128, 128], f32, tag="cpack32")
            nc.sync.dma_start(cpack32[:], cpack32_d[:])
            jones32 = cpack32[:, 0:128]    # all-ones (f32)
            qkbp = const.tile([128, L, 2 * DCH], f32, tag="qkbp")
            nc.sync.dma_start(qkbp[:], qkbp_d[:])
            biasp = const.tile([128, L, 32], f32, tag="biasp")
            nc.sync.dma_start(biasp[:], biasp_d[:])
            # LN epsilons as [128,1] const APs (float act-bias needs an AP)
            eps0 = const.tile([128, 1], f32, tag="eps0")
            nc.vector.memset(eps0[:], 1e-6)
            eps1 = const.tile([128, 1], f32, tag="eps1")
            nc.vector.memset(eps1[:], 1e-5)

            # residual, feature-major [128p, chunk, token], fp32
            h = hres.tile([128, DCH, M], f32, tag="h")
            nc.sync.dma_start(h[:], x_d[:].rearrange("c p m -> p c m"))

            def layer_norm(l, site):
                """Returns xln_bf [128, DCH, M] bf16.

                site 0: collapsed double-LN (no-affine eps=1e-6 then affine
                eps=1e-5)  -> rsqrt(v*(1+1e-5) + 1.00001e-6), then *g+b.
                site 1: plain affine LN eps=1e-5 -> rsqrt(v + 1e-5).
                """
                # column sums (fp32 matmul, no cast needed) and
                # sum-of-squares (bf16 via ACT Square) via all-ones matmuls
                ps_s = ps_a.tile([128, M], f32, tag="a")
                ps_q = ps_a.tile([128, M], f32, tag="a")
                for c in range(DCH):
                    nc.tensor.matmul(ps_s[:], jones32, h[:, c, :],
                                     start=(c == 0), stop=(c == DCH - 1))
                for c in range(DCH):
                    hsq = sq.tile([128, M], bf16, tag="sq")
                    nc.scalar.activation(hsq[:], h[:, c, :], Act.Square)
                    nc.tensor.matmul(ps_q[:], jones, hsq[:],
                                     start=(c == 0), stop=(c == DCH - 1))
                mu = stats.tile([128, M], f32, tag="st")
                nc.vector.tensor_scalar_mul(mu[:], ps_s[:], 1.0 / D)
                mu2 = stats.tile([128, M], f32, tag="st")
                nc.vector.tensor_mul(mu2[:], mu[:], mu[:])
                v = stats.tile([128, M], f32, tag="st")
                # v = ps_q/D - mu^2  (one fused op)
                nc.vector.scalar_tensor_tensor(v[:], ps_q[:], 1.0 / D, mu2[:],
                                               Alu.mult, Alu.subtract)
                # site 0 collapsed double-LN: rsqrt(v*(1+1e-5) + 1.00001e-6)
                #   = rsqrt(v + 1e-6) / sqrt(1+1e-5); the 1/sqrt(1+1e-5) is
                #   folded into the host-side gain g.
                alpha = stats.tile([128, M], f32, tag="st")
                sd = stats.tile([128, M], f32, tag="st")
                nc.scalar.activation(sd[:], v[:], Act.Sqrt,
                                     bias=(eps0 if site == 0 else eps1)[:])
                nc.vector.reciprocal_approx_fast(alpha[:], sd[:])
                xln = actb.tile([128, DCH, M], bf16, tag="xln")
                for cs in ((0, 1), (1, 3), (3, 5), (5, 7), (7, 8)):
                    c0, c1 = cs
                    w = c1 - c0
                    tt_full = sq.tile([128, 2, M], f32, tag="sqf")
                    t = tt_full[:, :w, :]
                    mu_b = mu[:].unsqueeze(1).to_broadcast([128, w, M])
                    al_b = alpha[:].unsqueeze(1).to_broadcast([128, w, M])
                    nc.vector.tensor_sub(t[:], h[:, c0:c1, :], mu_b)
                    nc.vector.tensor_mul(xln[:, c0:c1, :], t[:], al_b)
                return xln

            wq_pool = tfs.enter_context(tc.tile_pool(name="wq", bufs=1))
            wo_pool = tfs.enter_context(tc.tile_pool(name="wo", bufs=1))
            wf1_pool = tfs.enter_context(tc.tile_pool(name="wf1", bufs=1))
            wf2_pool = tfs.enter_context(tc.tile_pool(name="wf2", bufs=1))

            for l in range(L):
                # ---- attn pre-LN (collapsed double LN)
                xln = layer_norm(l, 0)

                wq = wq_pool.tile([128, DCH, 3 * INNER], bf16, tag="wq")
                for c in range(DCH):
                    nc.sync.dma_start(wq[:, c, :], wq_d[l, c])

                # ---- Q,K feature-major GEMM: out [2*INNER, M]
                qk = qkp.tile([128, 2 * DCH, M], bf16, tag="qk")
                for np_ in range(DCH):
                    ps = ps_a.tile([128, 2, M], f32, tag="a")
                    for i in range(2):
                        n = 2 * np_ + i
                        for c in range(DCH):
                            nc.tensor.matmul(ps[:, i, :],
                                             wq[:, c, n * 128:(n + 1) * 128],
                                             xln[:, c, :],
                                             start=(c == 0),
                                             stop=(c == DCH - 1))
                    for i in range(2):
                        n = 2 * np_ + i
                        nc.scalar.activation(qk[:, n, :], ps[:, i, :],
                                             Act.Identity,
                                             bias=qkbp[:, l, n:n + 1])

                # ---- V token-major GEMM: out [M, INNER]
                vtok = vtokp.tile([128, 2, INNER], bf16, tag="vtok")
                for mc in range(2):
                    for ns in range(2):
                        psv = ps_v.tile([128, 512], f32, tag="v")
                        for c in range(DCH):
                            nc.tensor.matmul(
                                psv[:],
                                xln[:, c, mc * 128:(mc + 1) * 128],
                                wq[:, c, 2 * INNER + ns * 512:
                                   2 * INNER + (ns + 1) * 512],
                                start=(c == 0), stop=(c == DCH - 1))
                        nc.scalar.activation(
                            vtok[:, mc, ns * 512:(ns + 1) * 512], psv[:],
                            Act.Copy)

                # ---- attention, head-pair by head-pair
                obuf = obufp.tile([128, DCH, M], bf16, tag="obuf")
                for hc in range(DCH):
                    # head pair (2*hc, 2*hc+1): even at partitions 0:64,
                    # odd at 64:128 -> S matmuls interleave across row
                    # groups so the PE array runs them concurrently.
                    es = []
                    pss = []
                    for j in range(2):
                        e_j = attp.tile([128, 2, M], bf16, tag="e",
                                        name=f"e_{l}_{hc}_{j}")
                        ps_j = ps_a.tile([128, 2, M], f32, tag="a",
                                         name=f"pss_{l}_{hc}_{j}")
                        es.append(e_j)
                        pss.append(ps_j)
                    for i in range(2):
                        for j in range(2):
                            hp = j * 64
                            nc.tensor.matmul(
                                pss[j][:, i, :],
                                qk[hp:hp + 64, DCH + hc,
                                   i * 128:(i + 1) * 128],
                                qk[hp:hp + 64, hc, :],
                                start=True, stop=False)
                        for j in range(2):
                            nc.tensor.matmul(pss[j][:, i, :], mrow, mcols[i],
                                             start=False, stop=True)
                    for j in range(2):
                        nc.scalar.activation(es[j][:], pss[j][:], Act.Exp,
                                             scale=0.125)
                    for j in range(2):
                        hh = 2 * hc + j
                        hp = j * 64
                        e = es[j]
                        ps_dn = ps_a.tile([128, M], f32, tag="a")
                        for i in range(2):
                            nc.tensor.matmul(ps_dn[:], jones, e[:, i, :],
                                             start=(i == 0), stop=(i == 1))
                        rd = stats.tile([128, M], f32, tag="st")
                        nc.vector.reciprocal_approx_fast(rd[0:64, :],
                                                         ps_dn[0:64, :])
                        ps_o = ps_a.tile([128, M], f32, tag="a")
                        for i in range(2):
                            nc.tensor.matmul(
                                ps_o[hp:hp + 64, :],
                                vtok[:, i, hh * 64:(hh + 1) * 64],
                                e[:, i, :],
                                start=(i == 0), stop=(i == 1),
                                tile_position=(0, hp))
                        nc.vector.tensor_tensor(
                            obuf[hp:hp + 64, hc, :], ps_o[hp:hp + 64, :],
                            rd[0:64, :], Alu.mult)
                # ---- output projection + residual + out_b
                wo = wo_pool.tile([128, DCH, D], bf16, tag="wo")
                nc.sync.dma_start(wo[:], wo_d[l].rearrange("c p n -> p c n"))
                for n in range(DCH):
                    ps = ps_a.tile([128, M], f32, tag="a")
                    for c in range(DCH):
                        nc.tensor.matmul(ps[:],
                                         wo[:, c, n * 128:(n + 1) * 128],
                                         obuf[:, c, :],
                                         start=(c == 0), stop=(c == DCH - 1))
                    nc.vector.scalar_tensor_tensor(
                        h[:, n, :], ps[:], biasp[:, l, n:n + 1], h[:, n, :],
                        Alu.add, Alu.add)

                # ---- ff pre-LN
                xln2 = layer_norm(l, 1)

                # ---- ff1 + gelu(x + b1)
                wf1 = wf1_pool.tile([128, DCH, FF], bf16, tag="wf1")
                nc.sync.dma_start(wf1[:], wf1_d[l].rearrange("c p n -> p c n"))
                g1 = g1p.tile([128, FCH, M], bf16, tag="g1")
                for n in range(FCH):
                    ps = ps_a.tile([128, M], f32, tag="a")
                    for c in range(DCH):
                        nc.tensor.matmul(ps[:],
                                         wf1[:, c, n * 128:(n + 1) * 128],
                                         xln2[:, c, :],
                                         start=(c == 0), stop=(c == DCH - 1))
                    nc.scalar.activation(g1[:, n, :], ps[:], Act.Gelu,
                                         bias=biasp[:, l, 8 + n:9 + n])

                # ---- ff2 + residual + b2
                wf2 = wf2_pool.tile([128, FCH, D], bf16, tag="wf2")
                nc.sync.dma_start(wf2[:], wf2_d[l].rearrange("c p n -> p c n"))
                for n in range(DCH):
                    ps = ps_a.tile([128, M], f32, tag="a")
                    for c in range(FCH):
                        nc.tensor.matmul(ps[:],
                                         wf2[:, c, n * 128:(n + 1) * 128],
                                         g1[:, c, :],
                                         start=(c == 0), stop=(c == FCH - 1))
                    nc.vector.scalar_tensor_tensor(
                        h[:, n, :], ps[:], biasp[:, l, 24 + n:25 + n],
                        h[:, n, :], Alu.add, Alu.add)

            # ---------------- AllGather x_comb ----------------
            # local contribution: columns m = b*32 + t for t in {0,1}
            x16 = const.tile([128, DCH, 2, B_LOC], bf16, tag="x16")
            src = h[:].rearrange("p c (b tt) -> p c tt b", b=B_LOC)[:, :, 0:2, :]
            nc.vector.tensor_copy(x16[:], src)
            tfs.close()   # free transformer pools for the heads stage
            cc_in = dram.tile([128, 128], bf16)
            nc.sync.dma_start(cc_in[:], x16[:].rearrange("p c tt b -> p (c tt b)"))
            cc_out = dram.tile([N_CORES * 128, 128], bf16)
            nc.gpsimd.collective_compute(
                "AllGather", Alu.bypass,
                replica_groups=[list(range(N_CORES))],
                ins=[cc_in[:].opt()], outs=[cc_out[:].opt()])
            nc.sync.dma_start(xcg_d[:], cc_out[:])

            gsb = const.tile([128, N_CORES, 128], bf16, tag="gsb")
            nc.sync.dma_start(
                gsb[:], cc_out[:].rearrange("(j p) f -> p j f", p=128))
            # PE warm-up after the long AllGather idle gap: a few throwaway
            # matmuls on the freshly-reloaded buffer re-arm the HAM clock
            # before the timing-critical head GEMMs.
            ps_w = ps_a.tile([128, 2, M], f32, tag="a", name="warmup_ps")
            for wi in range(16):
                nc.tensor.matmul(ps_w[:, wi % 2, :],
                                 gsb[:, wi % N_CORES, :],
                                 gsb[:].rearrange("p j f -> p (j f)")[:, 0:M],
                                 start=(wi < 2), stop=(wi >= 14))

            # build lhsT x_comb^T [2D, B] as bf16 [128, 16, 64]
            # gsb free layout per core j: (c, tt, b); feature chunk kc of
            # x_comb^T = tt*DCH + c
            xcombT = const.tile([128, 2 * DCH, B], bf16, tag="xcombT")
            for kc in range(2 * DCH):
                tt, c = kc // DCH, kc % DCH
                nc.vector.tensor_copy(
                    xcombT[:, kc, :],
                    gsb[:, :, c * 16 + tt * 8: c * 16 + tt * 8 + 8])

            # ---------------- 26 (padded 32) MLP heads ----------------
            hbb = const.tile([64, H_SLOTS, 2 * IDH], bf16, tag="hbb")
            import concourse.bass as bass_mod
            hb_bcast = bass_mod.AP(
                tensor=hbrow_d[:].tensor, offset=hbrow_d[:].offset,
                ap=[[0, 64]] + hbrow_d[:].ap[1:])
            nc.sync.dma_start(hbb[:], hb_bcast)
            hbp = const.tile([128, H_SLOTS, 1], f32, tag="hbp")
            nc.sync.dma_start(hbp[:], hbp_d[:])
            hw3 = const.tile([128, H_SLOTS * 4], bf16, tag="hw3")
            nc.sync.dma_start(hw3[:], hw3p_d[:])
            outacc = const.tile([64, H_SLOTS], f32, tag="outacc")

            w1h_pool = ctx.enter_context(tc.tile_pool(name="w1h", bufs=2))
            w2h_pool = ctx.enter_context(tc.tile_pool(name="w2h", bufs=2))
            hact = ctx.enter_context(tc.tile_pool(name="hact", bufs=2))

            # prefetch head weights (independent of the AllGather)
            w1h_tiles, w2h_tiles = [], []
            for n in range(H_SLOTS):
                w1h = w1h_pool.tile([128, 2 * DCH, IDH], bf16, tag="w1h")
                nc.sync.dma_start(w1h[:],
                                  hw1_d[n].rearrange("c p n2 -> p c n2"))
                w2h = w2h_pool.tile([128, 4, IDH], bf16, tag="w2h")
                nc.sync.dma_start(w2h[:],
                                  hw2_d[n].rearrange("c p n2 -> p c n2"))
                w1h_tiles.append(w1h)
                w2h_tiles.append(w2h)

            for n in range(H_SLOTS):
                w1h = w1h_tiles[n]
                w2h = w2h_tiles[n]
                ps1 = ps_a.tile([64, IDH], f32, tag="a")
                for kc in range(2 * DCH):
                    nc.tensor.matmul(ps1[:], xcombT[:, kc, :], w1h[:, kc, :],
                                     start=(kc == 0), stop=(kc == 2 * DCH - 1))
                t1 = hact.tile([64, IDH], f32, tag="ht")
                nc.vector.tensor_tensor(t1[:], ps1[:], hbb[:, n, 0:IDH],
                                        Alu.add)
                h1 = hact.tile([64, IDH], bf16, tag="hb")
                nc.vector.tensor_relu(h1[:], t1[:])
                # transpose h1 -> [IDH, 64]
                h1t = hact.tile([128, 4, 64], bf16, tag="h1t")
                for j in range(4):
                    pst = ps_a.tile([128, 64], bf16, tag="a")
                    nc.tensor.transpose(pst[:], h1[:, j * 128:(j + 1) * 128],
                                        ident[0:64, 0:64])
                    nc.vector.tensor_copy(h1t[:, j, :], pst[:])
                ps2 = ps_a.tile([64, IDH], f32, tag="a")
                for kc in range(4):
                    nc.tensor.matmul(ps2[:], h1t[:, kc, :], w2h[:, kc, :],
                                     start=(kc == 0), stop=(kc == 3))
                t2 = hact.tile([64, IDH], f32, tag="ht")
                nc.vector.tensor_tensor(t2[:], ps2[:], hbb[:, n, IDH:2 * IDH],
                                        Alu.add)
                h2 = hact.tile([64, IDH], bf16, tag="hb")
                nc.vector.tensor_relu(h2[:], t2[:])
                h2t = hact.tile([128, 4, 64], bf16, tag="h2t")
                for j in range(4):
                    pst = ps_a.tile([128, 64], bf16, tag="a")
                    nc.tensor.transpose(pst[:], h2[:, j * 128:(j + 1) * 128],
                                        ident[0:64, 0:64])
                    nc.vector.tensor_copy(h2t[:, j, :], pst[:])
                ps3 = ps_a.tile([64, 1], f32, tag="a")
                for kc in range(4):
                    nc.tensor.matmul(ps3[:], h2t[:, kc, :],
                                     hw3[:, n * 4 + kc:n * 4 + kc + 1],
                                     start=(kc == 0), stop=(kc == 3))
                nc.vector.tensor_scalar_add(outacc[:, n:n + 1], ps3[:],
                                            hbp[0:64, n, :])

            nc.sync.dma_start(out_d[:], outacc[:])

    nc.finalize()
    return nc


# ---------------------------------------------------------------- host side

def _head_map():
    """global head g -> (core, slot); core = g % 8, slot = g // 8."""
    m = []
    for n in range(H_SLOTS):
        for c in range(N_CORES):
            m.append(n * N_CORES + c)  # slot-major global index
    return m


def _prep_in_maps(inputs):
    x = np.asarray(inputs['x'], np.float32)
    qkv_w = np.asarray(inputs['qkv_w'], np.float32)
    out_w = np.asarray(inputs['out_w'], np.float32)
    out_b = np.asarray(inputs['out_b'], np.float32)
    attn_ln_g = np.asarray(inputs['attn_ln_g'], np.float32)
    attn_ln_b = np.asarray(inputs['attn_ln_b'], np.float32)
    ff_ln_g = np.asarray(inputs['ff_ln_g'], np.float32)
    ff_ln_b = np.asarray(inputs['ff_ln_b'], np.float32)
    ff_w1 = np.asarray(inputs['ff_w1'], np.float32)
    ff_b1 = np.asarray(inputs['ff_b1'], np.float32)
    ff_w2 = np.asarray(inputs['ff_w2'], np.float32)
    ff_b2 = np.asarray(inputs['ff_b2'], np.float32)
    head_w1 = np.asarray(inputs['head_w1'], np.float32)
    head_b1 = np.asarray(inputs['head_b1'], np.float32)
    head_w2 = np.asarray(inputs['head_w2'], np.float32)
    head_b2 = np.asarray(inputs['head_b2'], np.float32)
    head_w3 = np.asarray(inputs['head_w3'], np.float32)
    head_b3 = np.asarray(inputs['head_b3'], np.float32)

    # Fold the LN affine transform into the following GEMM weights (exact):
    #   xln = (h-mu)*alpha_hat;  y = (xln*g + b) @ W = xln @ (diag(g) W) + b@W
    # The collapsed double-LN 1/sqrt(1+1e-5) factor is folded into g too.
    ag_eff = attn_ln_g * np.float32((1.0 + 1e-5) ** -0.5)   # [L, D]
    qkvb = np.einsum('ld,ldn->ln', attn_ln_b, qkv_w)        # [L, 3*INNER]
    ff_b1 = ff_b1 + np.einsum('ld,ldn->ln', ff_ln_b, ff_w1)
    qkv_w = qkv_w * ag_eff[:, :, None]
    ff_w1 = ff_w1 * ff_ln_g[:, :, None]
    # V's LN-bias contribution passes through softmax unchanged (weights sum
    # to 1), so it folds into the output-projection bias exactly.
    vbias = qkvb[:, 2 * INNER:]                              # [L, INNER]
    out_b = out_b + np.einsum('lk,lkd->ld', vbias, out_w)

    # shared (replicated) tensors
    wq = np.ascontiguousarray(
        qkv_w.reshape(L, DCH, 128, 3 * INNER)).astype(BF16)
    wo = np.ascontiguousarray(out_w.reshape(L, DCH, 128, D)).astype(BF16)
    wf1 = np.ascontiguousarray(ff_w1.reshape(L, DCH, 128, FF)).astype(BF16)
    wf2 = np.ascontiguousarray(ff_w2.reshape(L, FCH, 128, D)).astype(BF16)

    biasp = np.zeros((128, L, 32), np.float32)
    biasp[:, :, 0:8] = out_b.reshape(L, 8, 128).transpose(2, 0, 1)
    biasp[:, :, 8:24] = ff_b1.reshape(L, 16, 128).transpose(2, 0, 1)
    biasp[:, :, 24:32] = ff_b2.reshape(L, 8, 128).transpose(2, 0, 1)

    qkbp = np.ascontiguousarray(
        qkvb[:, :2 * INNER].reshape(L, 2 * DCH, 128).transpose(2, 0, 1))

    cpack = np.zeros((128, 1024), np.float32)
    cpack[:, 0:128] = np.eye(128, dtype=np.float32)
    cpack[:, 128:384] = 1.0
    # rank-5 additive attention mask: M_i = -800*J + 800*sum_bl u_bl (x) v_{4i+bl}
    # (applied inside the S-matmul PSUM accumulation; exp(0.125*-800) == 0)
    cpack[0, 384:512] = 1.0                       # ones row of lhsT
    for bl in range(4):
        cpack[1 + bl, 384 + 32 * bl:384 + 32 * (bl + 1)] = 1.0   # u_bl
    for i in range(2):
        base = 512 + 256 * i
        cpack[0, base:base + 256] = -800.0
        for bl in range(4):
            bk = 4 * i + bl
            cpack[1 + bl, base + 32 * bk:base + 32 * (bk + 1)] = 800.0
    cpack = cpack.astype(BF16)
    cpack32 = np.ones((128, 128), np.float32)

    in_maps = []
    for c in range(N_CORES):
        xs = x[c * B_LOC:(c + 1) * B_LOC].reshape(M, D)  # [256, 1024]
        x_fm = np.ascontiguousarray(xs.T.reshape(DCH, 128, M))

        hw1 = np.zeros((H_SLOTS, 2 * DCH, 128, IDH), np.float32)
        hw2 = np.zeros((H_SLOTS, 4, 128, IDH), np.float32)
        hw3p = np.zeros((128, H_SLOTS * 4), np.float32)
        hbrow = np.zeros((1, H_SLOTS, 2 * IDH), np.float32)
        hbp = np.zeros((128, H_SLOTS, 1), np.float32)
        for n in range(H_SLOTS):
            g = n * N_CORES + c
            if g >= NOUT:
                continue
            hw1[n] = head_w1[g].reshape(2 * DCH, 128, IDH)
            hw2[n] = head_w2[g].reshape(4, 128, IDH)
            hw3p[:, n * 4:(n + 1) * 4] = head_w3[g].reshape(4, 128).T
            hbrow[0, n, 0:IDH] = head_b1[g]
            hbrow[0, n, IDH:2 * IDH] = head_b2[g]
            hbp[:, n, 0] = head_b3[g, 0]
        in_maps.append({
            'x': x_fm,
            'wq': wq, 'wo': wo, 'wf1': wf1, 'wf2': wf2,
            'biasp': biasp, 'qkbp': qkbp,
            'cpack': cpack, 'cpack32': cpack32,
            'hw1': hw1.astype(BF16), 'hw2': hw2.astype(BF16),
            'hw3p': hw3p.astype(BF16), 'hbrow': hbrow.astype(BF16),
            'hbp': hbp,
        })
    return in_maps


def _get_nc():
    if 'nc' not in _CACHE:
        _CACHE['nc'] = _build_nc()
    return _CACHE['nc']


def kernel(**inputs):
    from concourse.bass_utils import run_bass_kernel_spmd
    nc = _get_nc()
    in_maps = _prep_in_maps(inputs)
    res = run_bass_kernel_spmd(nc, in_maps, core_ids=list(range(N_CORES)))
    out = np.zeros((B, NOUT, 1), np.float32)
    for c in range(N_CORES):
        oh = res.results[c]['out_h']       # [64, H_SLOTS]
        for n in range(H_SLOTS):
            g = n * N_CORES + c
            if g < NOUT:
                out[:, g, 0] = oh[:, n]
    return out
